# revision 1
# baseline (speedup 1.0000x reference)
"""Trainium2 Bass kernel for DualHazardHead (moe_routing).

Computation per token t:
  x = concat(h, a, d, age)            [594]
  z = gelu(x @ Wt + bt)               [256]
  pw = softmax(h @ Wr + br)           [7]
  inst  = z @ Wbi + bbi + sum_p pw_p (z @ Wei_p + bei_p)   [20]
  group = z @ Wbg + bbg + sum_p pw_p (z @ Weg_p + beg_p)   [20]

Sharding: pure data parallel over B (32 -> 4 per core) on 8 NeuronCores.

On-device layout strategy (per core, NTOK=8192 tokens, 16 macro tiles of 512):
  - x arrives token-major; PE transpose (matmul w/ identity) produces
    xT [feature, token] blocks; a constant-ones row is injected at
    partition 82 of feature-block 4 so the router bias rides the matmul.
  - trunk out zT [256, tok] in PSUM -> exact GELU on ACT (bias fused).
  - router out pwT [7, tok]; PE transpose to token-side [tok, 7];
    softmax exp computed as (1+tanh(l/2))/(1-tanh(l/2)) so GELU and the
    softmax share ONE ACT table set (no ~2.7us table reloads).
  - heads: E [128tok, 320] PSUM; columns laid out c=(h*20+k)*8+p with
    p in 0..6 = experts, p=7 = base head; biases accumulated via a K=1
    ones-row matmul. Combine = one broadcast multiply by pw8 (softmax
    weights with a 1.0 in slot 7) + one strided reduce over p.
"""

import os

import numpy as np

B, T = 32, 2048
HID, ACTD, SRC, AGE = 512, 64, 2, 16
TRUNK, BINS, PHASES = 256, 20, 7
IN_DIM = HID + ACTD + SRC + AGE  # 594
NCORES = 8
B_LOC = B // NCORES  # 4
NTOK = B_LOC * T  # 8192
MACRO = 512
NMACRO = NTOK // MACRO  # 16
SUB = MACRO // 128  # 4
NHK = 2 * BINS  # 40 (head, bin) pairs
NP8 = PHASES + 1  # 7 experts + 1 base slot
NCOL = NHK * NP8  # 320 head-matmul output columns
KBLK = [128, 128, 128, 128, 82]  # xT k-block sizes (594 features)

_BUILT = {}
LAST_RESULT = None


def _build_module():
    """Build the Bass module (same NEFF for all cores)."""
    import concourse.bass as bass
    import concourse.tile as tile
    from concourse import bacc, mybir
    from concourse.masks import make_identity

    f32 = mybir.dt.float32
    # Matmul-operand dtype: float32r streams 1 row/cycle on the PE (vs 4
    # cycles/row for plain fp32) at TF32-like precision.
    mmdt = {"f32": f32, "f32r": mybir.dt.float32r, "bf16": mybir.dt.bfloat16}[
        os.environ.get("KERNEL_MM_DT", "f32r")
    ]

    def M(ap):
        return ap
    AF = mybir.ActivationFunctionType
    ALU = mybir.AluOpType
    ts = bass.ts

    nc = bacc.Bacc("TRN2", target_bir_lowering=False, debug=False)

    x_d = nc.dram_tensor("x", [NTOK, IN_DIM], mmdt, kind="ExternalInput")
    wt_d = nc.dram_tensor("wt", [128, 5, TRUNK], mmdt, kind="ExternalInput")
    wr_d = nc.dram_tensor("wr", [128, 5, PHASES], mmdt, kind="ExternalInput")
    wh_d = nc.dram_tensor("wh", [128, 2, NCOL], mmdt, kind="ExternalInput")
    dr_d = nc.dram_tensor("dr", [1, NCOL], mmdt, kind="ExternalInput")
    tb_d = nc.dram_tensor("tb", [128, 2], f32, kind="ExternalInput")
    rb_d = nc.dram_tensor("rb", [PHASES, 1], f32, kind="ExternalInput")
    inst_d = nc.dram_tensor("inst", [NTOK, BINS], f32, kind="ExternalOutput")
    grp_d = nc.dram_tensor("grp", [NTOK, BINS], f32, kind="ExternalOutput")

    xv = x_d[:, :].rearrange("(m s p) f -> m p s f", p=128, s=SUB)
    iv = inst_d[:, :].rearrange("(m s p) k -> m p s k", p=128, s=SUB)
    gv = grp_d[:, :].rearrange("(m s p) k -> m p s k", p=128, s=SUB)

    with tile.TileContext(nc) as tc:
        with (
            tc.tile_pool(name="const", bufs=1) as const,
            tc.tile_pool(name="xin", bufs=2) as xin,
            tc.tile_pool(name="xt", bufs=2) as xtp,
            tc.tile_pool(name="zs", bufs=2) as zsp,
            tc.tile_pool(name="sm", bufs=2) as smp,
            tc.tile_pool(name="prod", bufs=3) as prodp,
            tc.tile_pool(name="outp", bufs=2) as outp,
            tc.tile_pool(name="ps_xt", bufs=3, space="PSUM") as ps_xt,
            tc.tile_pool(name="ps_z", bufs=2, space="PSUM") as ps_z,
            tc.tile_pool(name="ps_pw", bufs=1, space="PSUM") as ps_pw,
            tc.tile_pool(name="ps_e", bufs=2, space="PSUM") as ps_e,
        ):
            ident_f = const.tile([128, 128], f32)
            make_identity(nc, ident_f)
            ident = const.tile([128, 128], mmdt)
            nc.vector.tensor_copy(out=ident, in_=ident_f)
            ones_f = const.tile([1, 128], f32)
            nc.gpsimd.memset(ones_f, 1.0)
            ones1 = const.tile([1, 128], mmdt)
            nc.vector.tensor_copy(out=ones1, in_=ones_f)
            wt = const.tile([128, 5, TRUNK], mmdt)
            nc.gpsimd.dma_start(wt, wt_d[:])
            wr = const.tile([128, 5, PHASES], mmdt)
            nc.gpsimd.dma_start(wr, wr_d[:])
            wh = const.tile([128, 2, NCOL], mmdt)
            nc.gpsimd.dma_start(wh, wh_d[:])
            dr = const.tile([1, NCOL], mmdt)
            nc.gpsimd.dma_start(dr, dr_d[:])
            tb = const.tile([128, 2], f32)
            nc.gpsimd.dma_start(tb, tb_d[:])
            rb = const.tile([PHASES, 1], f32)
            nc.gpsimd.dma_start(rb, rb_d[:])

            # Persistent double-buffered xT tiles (manual double buffer so
            # slot-release ticks stay on engines the PE already observes).
            xts = [
                const.tile([128, 5, MACRO], mmdt, name=f"xtbuf{i}")
                for i in range(2)
            ]

            # PE prewarm: consume each const via a dummy transpose so later
            # real PE instructions never need a startup semaphore wait
            # (each dummy carries exactly one wait).
            pdum = ps_xt.tile([128, MACRO], mmdt, tag="pxt")
            nc.tensor.transpose(M(pdum[:, 0:128]), M(ident), M(ident))
            nc.tensor.transpose(M(pdum[:, 0:128]), M(wt[:, 0, 0:128]), M(ident))
            nc.tensor.transpose(M(pdum[:7, 0:128]), M(wr[:, 0, :]), M(ident))
            nc.tensor.transpose(M(pdum[:, 0:128]), M(wh[:, 0, 0:128]), M(ident))
            if mmdt == mybir.dt.float32r:
                # K=1 f32r transposes are ISA-invalid; run these two dummies
                # through the plain-f32 path (same bits, nobody reads them).
                nc.tensor.transpose(
                    pdum[:, 0:1].bitcast(f32), dr[:1, 0:128].bitcast(f32),
                    ident_f[:1, :1],
                )
                nc.tensor.transpose(
                    pdum[:, 0:1].bitcast(f32), ones1[:, :].bitcast(f32),
                    ident_f[:1, :1],
                )
            else:
                nc.tensor.transpose(pdum[:1, 0:1], dr[:1, 0:1], ident[:1, :1])
                nc.tensor.transpose(pdum[:1, 0:1], ones1[:1, 0:1], ident[:1, :1])

            def transp_block(x_t, xt, b):
                w_b = 128 if b < 4 else IN_DIM - 512  # 82
                pxt = ps_xt.tile([128, MACRO], mmdt, tag="pxt")
                for s in range(SUB):
                    nc.tensor.transpose(
                        M(pxt[:w_b, ts(s, 128)]),
                        M(x_t[:, s, b * 128 : b * 128 + w_b]),
                        M(ident),
                    )
                # PSUM -> SBUF copy (b=1 on DVE to balance engine load)
                if b == 1:
                    nc.vector.tensor_copy(out=xt[:w_b, b, :], in_=pxt[:w_b, :])
                else:
                    nc.scalar.copy(out=xt[:w_b, b, :], in_=pxt[:w_b, :])

            for m in range(NMACRO):
                # ---- load x (token-major) ----
                x_t = xin.tile([128, SUB, IN_DIM], mmdt)
                for s in range(SUB):
                    nc.sync.dma_start(x_t[:, s, :], xv[m, :, s, :])
                xt = xts[m % 2]

                pz0 = ps_z.tile([128, MACRO], f32, tag="pz")
                pz1 = ps_z.tile([128, MACRO], f32, tag="pz")

                def trunk_mm(b):
                    kb = KBLK[b]
                    nc.tensor.matmul(
                        pz0, M(wt[:kb, b, 0:128]), M(xt[:kb, b, :]),
                        start=(b == 0), stop=(b == 4),
                    )
                    nc.tensor.matmul(
                        pz1, M(wt[:kb, b, 128:256]), M(xt[:kb, b, :]),
                        start=(b == 0), stop=(b == 4),
                    )

                # Weave: trunk_mm(b) waits on copy(b); emitting it before
                # transp(b+2) lets the PE observe the copy engine's clock so
                # the PSUM-slot reuse needs no extra semaphore wait on the
                # transposes (avoids event-semaphore indirection).
                transp_block(x_t, xt, 0)
                transp_block(x_t, xt, 1)
                trunk_mm(0)
                transp_block(x_t, xt, 2)
                trunk_mm(1)
                transp_block(x_t, xt, 3)
                trunk_mm(2)
                transp_block(x_t, xt, 4)
                trunk_mm(3)
                trunk_mm(4)

                # ---- router matmuls: pwT [7, 512] (h = blocks 0..3) ----
                ppw = ps_pw.tile([128, MACRO], f32, tag="ppw")
                for b in range(4):
                    nc.tensor.matmul(
                        ppw[:PHASES], M(wr[:128, b, :]), M(xt[:128, b, :]),
                        start=(b == 0), stop=(b == 3),
                    )

                # ---- GELU (exact) with fused trunk bias ----
                zs = zsp.tile([128, 2, MACRO], mmdt)
                nc.scalar.activation(
                    out=zs[:, 0, :], in_=pz0, func=AF.Gelu,
                    bias=tb[:, 0:1], scale=1.0,
                )
                nc.scalar.activation(
                    out=zs[:, 1, :], in_=pz1, func=AF.Gelu,
                    bias=tb[:, 1:2], scale=1.0,
                )

                # ---- router logits to token-side + softmax via tanh ----
                # pwT copy on DVE: doubles as the PE<->DVE clock bridge so
                # the heads matmuls inherit DVE's combine-release ticks.
                pwt_sb = smp.tile([PHASES, MACRO], f32, tag="pwt")
                nc.vector.tensor_scalar_add(pwt_sb, ppw[:PHASES], rb)
                ppt = ps_pw.tile([128, SUB, PHASES], f32, tag="ppw")
                for s in range(SUB):
                    nc.tensor.transpose(
                        ppt[:, s, :], pwt_sb[:, ts(s, 128)],
                        ident_f[:PHASES, :PHASES],
                    )
                th = smp.tile([128, SUB, PHASES], f32, tag="th")
                nc.scalar.activation(out=th, in_=ppt, func=AF.Tanh, scale=0.5)
                den = smp.tile([128, SUB, PHASES], f32, tag="den")
                # den = 1 - t
                nc.vector.tensor_scalar(
                    out=den, in0=th, scalar1=-1.0, scalar2=1.0,
                    op0=ALU.mult, op1=ALU.add,
                )
                pw8 = smp.tile([128, SUB, NP8], f32, tag="pw8")
                nc.gpsimd.memset(pw8[:, :, PHASES : PHASES + 1], 1.0)
                # exp(l) = (1 + t) / (1 - t)
                nc.vector.reciprocal(out=den, in_=den)
                nc.vector.scalar_tensor_tensor(
                    out=pw8[:, :, :PHASES], in0=th, scalar=1.0, in1=den,
                    op0=ALU.add, op1=ALU.mult,
                )
                ssum = smp.tile([128, SUB], f32, tag="ssum")
                nc.vector.reduce_sum(
                    out=ssum, in_=pw8[:, :, :PHASES], axis=mybir.AxisListType.X
                )
                rec = smp.tile([128, SUB], f32, tag="rec")
                nc.vector.reciprocal(out=rec, in_=ssum)
                nc.vector.tensor_tensor(
                    out=pw8[:, :, :PHASES],
                    in0=pw8[:, :, :PHASES],
                    in1=rec[:, :, None].to_broadcast([128, SUB, PHASES]),
                    op=ALU.mult,
                )

                # ---- heads + combine per 128-token subtile ----
                osb = outp.tile([128, SUB, NHK], f32)
                for s in range(SUB):
                    pe = ps_e.tile([128, NCOL], f32)
                    nc.tensor.matmul(
                        pe, M(ones1[:1, :]), M(dr[:, :]), start=True, stop=False
                    )
                    nc.tensor.matmul(
                        pe, M(zs[:, 0, ts(s, 128)]), M(wh[:, 0, :]),
                        start=False, stop=False,
                    )
                    nc.tensor.matmul(
                        pe, M(zs[:, 1, ts(s, 128)]), M(wh[:, 1, :]),
                        start=False, stop=True,
                    )
                    prod = prodp.tile([128, NHK, NP8], f32)
                    nc.vector.tensor_tensor(
                        out=prod,
                        in0=pe.rearrange("p (hk e) -> p hk e", e=NP8),
                        in1=pw8[:, s : s + 1, :].to_broadcast([128, NHK, NP8]),
                        op=ALU.mult,
                    )
                    nc.vector.reduce_sum(
                        out=osb[:, s, :], in_=prod, axis=mybir.AxisListType.X
                    )

                nc.sync.dma_start(iv[m], osb[:, :, 0:BINS])
                nc.sync.dma_start(gv[m], osb[:, :, BINS:NHK])

    nc.compile()
    return nc


def _host_weights(inp):
    """Rearrange weights into on-device layouts (host-side, one-time)."""
    f = np.float32
    wt = np.zeros((128, 5, TRUNK), f)
    for b in range(4):
        wt[:, b, :] = inp["trunk_w"][b * 128 : (b + 1) * 128]
    wt[:82, 4, :] = inp["trunk_w"][512:IN_DIM]

    wr = np.zeros((128, 5, PHASES), f)
    for b in range(4):
        wr[:, b, :] = inp["router_w"][b * 128 : (b + 1) * 128]
    rb = np.ascontiguousarray(inp["router_b"].reshape(PHASES, 1))

    # heads: col c = (h*20+k)*8 + p ; p<7 experts, p=7 base
    wh_full = np.zeros((TRUNK, NHK, NP8), f)
    dr_full = np.zeros((NHK, NP8), f)
    wh_full[:, :BINS, :PHASES] = np.transpose(inp["inst_exp_w"], (1, 2, 0))
    wh_full[:, BINS:, :PHASES] = np.transpose(inp["group_exp_w"], (1, 2, 0))
    wh_full[:, :BINS, PHASES] = inp["inst_base_w"]
    wh_full[:, BINS:, PHASES] = inp["group_base_w"]
    dr_full[:BINS, :PHASES] = inp["inst_exp_b"].T
    dr_full[BINS:, :PHASES] = inp["group_exp_b"].T
    dr_full[:BINS, PHASES] = inp["inst_base_b"]
    dr_full[BINS:, PHASES] = inp["group_base_b"]
    wh = wh_full.reshape(TRUNK, NCOL).reshape(2, 128, NCOL).transpose(1, 0, 2).copy()
    dr = dr_full.reshape(1, NCOL).copy()

    tb = np.ascontiguousarray(inp["trunk_b"].reshape(2, 128).T)
    return wt, wr, wh, dr, tb, rb


def _patch_ldw_opt():
    """Enable walrus LDWEIGHTS pipelining (hides weight-load latency)."""
    import concourse.bass_utils as bu

    if getattr(bu, "_ldw_opt_patched", False):
        return
    orig = bu.run_command

    def patched(argv, **kw):
        argv = [
            "--enable-ldw-opt=true" if a == "--enable-ldw-opt=false" else a
            for a in argv
        ]
        return orig(argv, **kw)

    bu.run_command = patched
    bu._ldw_opt_patched = True


def kernel(**inputs):
    global LAST_RESULT
    import sys

    if "/opt/trn_rl_repo" not in sys.path:
        sys.path.insert(0, "/opt/trn_rl_repo")
    from concourse.bass_utils import run_bass_kernel_spmd

    if os.environ.get("KERNEL_LDW_OPT", "0") == "1":
        _patch_ldw_opt()

    inp = {k: np.asarray(v, dtype=np.float32 if np.asarray(v).dtype != np.int32 else np.int32) for k, v in inputs.items()}

    if "nc" not in _BUILT:
        _BUILT["nc"] = _build_module()
    nc = _BUILT["nc"]

    wt, wr, wh, dr, tb, rb = _host_weights(inp)

    x_full = np.concatenate(
        [inp["h_t"], inp["a_t"], inp["d_t"], inp["age_embed"]], axis=-1
    )  # [B, T, 594]

    if os.environ.get("KERNEL_MM_DT", "f32r") == "bf16":
        import ml_dtypes

        bf16 = ml_dtypes.bfloat16
        x_full = x_full.astype(bf16)
        wt, wr, wh, dr = (a.astype(bf16) for a in (wt, wr, wh, dr))

    in_maps = []
    for c in range(NCORES):
        xc = np.ascontiguousarray(
            x_full[c * B_LOC : (c + 1) * B_LOC].reshape(NTOK, IN_DIM)
        )
        in_maps.append(
            {"x": xc, "wt": wt, "wr": wr, "wh": wh, "dr": dr, "tb": tb, "rb": rb}
        )

    res = run_bass_kernel_spmd(nc, in_maps, core_ids=list(range(NCORES)))
    LAST_RESULT = res

    inst = np.empty((B, T, BINS), np.float32)
    grp = np.empty((B, T, BINS), np.float32)
    for c in range(NCORES):
        inst[c * B_LOC : (c + 1) * B_LOC] = res.results[c]["inst"].reshape(
            B_LOC, T, BINS
        )
        grp[c * B_LOC : (c + 1) * B_LOC] = res.results[c]["grp"].reshape(
            B_LOC, T, BINS
        )
    return inst, grp



# revision 10
# speedup vs baseline: 1.5083x; 1.5083x over previous
"""Trainium2 Bass kernel for DualHazardHead (moe_routing).

Computation per token t:
  x = concat(h, a, d, age)            [594]
  z = gelu(x @ Wt + bt)               [256]
  pw = softmax(h @ Wr + br)           [7]
  inst  = z @ Wbi + bbi + sum_p pw_p (z @ Wei_p + bei_p)   [20]
  group = z @ Wbg + bbg + sum_p pw_p (z @ Weg_p + beg_p)   [20]

Sharding: pure data parallel over B (32 -> 4 per core) on 8 NeuronCores.

v2 design (per core, NTOK=8192 tokens, 16 macro tiles of 512):
  - x is transposed on the HOST to feature-major [594, NTOK] (and cast to
    bf16), so the kernel DMAs xT blocks straight into SBUF: no on-device
    PE transposes, no PSUM->SBUF copies for x.
  - trunk zT [256, tok] in PSUM -> exact GELU on ACT (bias fused) -> bf16.
  - router logits pwT [7, tok] on PE; tanh(l/2 + rb/2) computed PHASE-major
    on ACT (router bias fused into the activation), then 4 small PE
    transposes to token-side; softmax exp via (1+t)/(1-t) on DVE so GELU
    and softmax share one ACT table set.
  - heads: pe [128tok, 320] PSUM; columns c=(h*20+k)*8+p with p in 0..6 =
    experts, p=7 = base head; biases accumulated via a K=1 ones-row matmul.
    Combine = one broadcast multiply by pw8 (slot 7 = 1.0) + strided
    reduce over p on DVE.
  - PE program order per macro: trunk(10) -> router(4) -> heads(12) ->
    softmax transposes(4), so no PE stall waits on the ACT tanh roundtrip.
"""

import os

import numpy as np

B, T = 32, 2048
HID, ACTD, SRC, AGE = 512, 64, 2, 16
TRUNK, BINS, PHASES = 256, 20, 7
IN_DIM = HID + ACTD + SRC + AGE  # 594
NCORES = 8
B_LOC = B // NCORES  # 4
NTOK = B_LOC * T  # 8192
MACRO = 512
NMACRO = NTOK // MACRO  # 16
SUB = MACRO // 128  # 4
NHK = 2 * BINS  # 40 (head, bin) pairs
NP8 = PHASES + 1  # 7 experts + 1 base slot
NCOL = NHK * NP8  # 320 head-matmul output columns
KBLK = [128, 128, 128, 128, 82]  # xT k-block sizes (594 features)

_BUILT = {}
LAST_RESULT = None


def _mm_dt_name():
    return os.environ.get("KERNEL_MM_DT", "bf16")


def _build_module():
    """Build the Bass module (same NEFF for all cores)."""
    import concourse.bass as bass
    import concourse.tile as tile
    from concourse import bacc, mybir
    from concourse.masks import make_identity

    f32 = mybir.dt.float32
    mmdt = {"f32": f32, "f32r": mybir.dt.float32r, "bf16": mybir.dt.bfloat16}[
        _mm_dt_name()
    ]

    AF = mybir.ActivationFunctionType
    ALU = mybir.AluOpType
    ts = bass.ts

    nc = bacc.Bacc("TRN2", target_bir_lowering=False, debug=False)

    xh_d = nc.dram_tensor("xh", [HID, NTOK], mmdt, kind="ExternalInput")
    xr_d = nc.dram_tensor("xr", [IN_DIM - HID, NTOK], mmdt, kind="ExternalInput")
    wt_d = nc.dram_tensor("wt", [128, 5, TRUNK], mmdt, kind="ExternalInput")
    wr_d = nc.dram_tensor("wr", [128, 4, PHASES], mmdt, kind="ExternalInput")
    wh_d = nc.dram_tensor("wh", [128, 2, NCOL], mmdt, kind="ExternalInput")
    dr_d = nc.dram_tensor("dr", [1, NCOL], mmdt, kind="ExternalInput")
    tb_d = nc.dram_tensor("tb", [128, 2], f32, kind="ExternalInput")
    rb2_d = nc.dram_tensor("rb2", [PHASES, 1], f32, kind="ExternalInput")
    out_d = nc.dram_tensor("out", [NTOK, NHK], f32, kind="ExternalOutput")

    xhv = xh_d[:, :].rearrange("(b p) (m t) -> m p b t", p=128, t=MACRO)
    xrv = xr_d[:, :].rearrange("q (m t) -> m q t", t=MACRO)
    ov = out_d[:, :].rearrange("(m s p) k -> m p s k", p=128, s=SUB)

    with tile.TileContext(nc) as tc:
        with (
            tc.tile_pool(name="const", bufs=1) as const,
            tc.tile_pool(name="xin", bufs=2) as xin,
            tc.tile_pool(name="zs", bufs=2) as zsp,
            tc.tile_pool(name="sm", bufs=2) as smp,
            tc.tile_pool(name="prod", bufs=3) as prodp,
            tc.tile_pool(name="outp", bufs=2) as outp,
            tc.tile_pool(name="ps_z", bufs=2, space="PSUM") as ps_z,
            tc.tile_pool(name="ps_pw", bufs=1, space="PSUM") as ps_pw,
            tc.tile_pool(name="ps_e", bufs=4, space="PSUM") as ps_e,
        ):
            ident_f = const.tile([128, 128], f32)
            make_identity(nc, ident_f)
            ident = const.tile([128, 128], mmdt)
            nc.vector.tensor_copy(out=ident, in_=ident_f)
            ones_f = const.tile([1, 128], f32)
            nc.gpsimd.memset(ones_f, 1.0)
            ones1 = const.tile([1, 128], mmdt)
            nc.vector.tensor_copy(out=ones1, in_=ones_f)
            wt = const.tile([128, 5, TRUNK], mmdt)
            nc.gpsimd.dma_start(wt, wt_d[:])
            wr = const.tile([128, 4, PHASES], mmdt)
            nc.gpsimd.dma_start(wr, wr_d[:])
            wh = const.tile([128, 2, NCOL], mmdt)
            nc.gpsimd.dma_start(wh, wh_d[:])
            dr = const.tile([1, NCOL], mmdt)
            nc.gpsimd.dma_start(dr, dr_d[:])
            tb = const.tile([128, 2], f32)
            nc.gpsimd.dma_start(tb, tb_d[:])
            rb2 = const.tile([PHASES, 1], f32)
            nc.gpsimd.dma_start(rb2, rb2_d[:])

            # PE prewarm: consume each const via a dummy transpose so later
            # real PE instructions never need a startup semaphore wait.
            # Dummies write into pool tiles that the loop reuses anyway.
            pwf = ps_pw.tile([128, MACRO], f32, tag="ppw")
            pe_w = ps_e.tile([128, NCOL], f32, tag="pe")
            if mmdt == f32:
                pwm = pe_w
            else:
                pwm = pe_w.bitcast(mmdt)
            nc.tensor.transpose(pwf[:7, 0:7], ident_f[:7, :7], ident_f[:7, :7])
            if mmdt == mybir.dt.float32r:
                nc.tensor.transpose(
                    pwm[:, 0:128].bitcast(f32), wt[:, 0, 0:128].bitcast(f32),
                    ident_f,
                )
                nc.tensor.transpose(
                    pwm[:7, 0:128].bitcast(f32), wr[:, 0, :].bitcast(f32),
                    ident_f,
                )
                nc.tensor.transpose(
                    pwm[:, 0:128].bitcast(f32), wh[:, 0, 0:128].bitcast(f32),
                    ident_f,
                )
                nc.tensor.transpose(
                    pwm[:, 0:1].bitcast(f32), dr[:1, 0:128].bitcast(f32),
                    ident_f[:1, :1],
                )
                nc.tensor.transpose(
                    pwm[:, 0:1].bitcast(f32), ones1[:, :].bitcast(f32),
                    ident_f[:1, :1],
                )
            else:
                nc.tensor.transpose(pwm[:, 0:128], wt[:, 0, 0:128], ident)
                nc.tensor.transpose(pwm[:7, 0:128], wr[:, 0, :], ident)
                nc.tensor.transpose(pwm[:, 0:128], wh[:, 0, 0:128], ident)
                nc.tensor.transpose(pwm[:1, 0:1], dr[:1, 0:1], ident[:1, :1])
                nc.tensor.transpose(pwm[:1, 0:1], ones1[:1, 0:1], ident[:1, :1])

            for m in range(NMACRO):
                # ---- load xT (feature-major, straight from host) ----
                xt = xin.tile([128, 5, MACRO], mmdt)
                nc.sync.dma_start(xt[:, 0:4, :], xhv[m])
                nc.sync.dma_start(xt[: KBLK[4], 4, :], xrv[m])

                # ---- trunk matmuls: zT [256, 512] over 2 PSUM halves ----
                pz0 = ps_z.tile([128, MACRO], f32, tag="pz")
                pz1 = ps_z.tile([128, MACRO], f32, tag="pz")
                for b in range(5):
                    kb = KBLK[b]
                    nc.tensor.matmul(
                        pz0, wt[:kb, b, 0:128], xt[:kb, b, :],
                        start=(b == 0), stop=(b == 4),
                    )
                    nc.tensor.matmul(
                        pz1, wt[:kb, b, 128:256], xt[:kb, b, :],
                        start=(b == 0), stop=(b == 4),
                    )

                # ---- router matmuls: pwT [7, 512] (h = blocks 0..3) ----
                ppw = ps_pw.tile([128, MACRO], f32, tag="ppw")
                for b in range(4):
                    nc.tensor.matmul(
                        ppw[:PHASES], wr[:128, b, :], xt[:128, b, :],
                        start=(b == 0), stop=(b == 3),
                    )

                # ---- GELU (exact) with fused trunk bias -> bf16 z ----
                zs = zsp.tile([128, 2, MACRO], mmdt)
                nc.scalar.activation(
                    out=zs[:, 0, :], in_=pz0, func=AF.Gelu,
                    bias=tb[:, 0:1], scale=1.0,
                )
                nc.scalar.activation(
                    out=zs[:, 1, :], in_=pz1, func=AF.Gelu,
                    bias=tb[:, 1:2], scale=1.0,
                )

                # ---- tanh(l/2 + rb/2) phase-major (router bias fused) ----
                thp = smp.tile([PHASES, MACRO], f32, tag="thp")
                nc.scalar.activation(
                    out=thp, in_=ppw[:PHASES], func=AF.Tanh,
                    bias=rb2, scale=0.5,
                )

                # ---- heads: pe [128tok, 320] per subtile ----
                osb = outp.tile([128, SUB, NHK], f32)
                pes = []
                for s in range(SUB):
                    pe = ps_e.tile([128, NCOL], f32, tag="pe")
                    pes.append(pe)
                    nc.tensor.matmul(
                        pe, ones1[:1, :], dr[:, :], start=True, stop=False
                    )
                    nc.tensor.matmul(
                        pe, zs[:, 0, ts(s, 128)], wh[:, 0, :],
                        start=False, stop=False,
                    )
                    nc.tensor.matmul(
                        pe, zs[:, 1, ts(s, 128)], wh[:, 1, :],
                        start=False, stop=True,
                    )

                # ---- tanh to token-side (after heads in PE queue) ----
                # ppt aliases the ppw bank: ppw is dead once thp is computed.
                ppt = ps_pw.tile([128, SUB, PHASES], f32, tag="ppw")
                for s in range(SUB):
                    nc.tensor.transpose(
                        ppt[:, s, :], thp[:, ts(s, 128)],
                        ident_f[:PHASES, :PHASES],
                    )

                # ---- softmax from tanh: exp(l) = (1+t)/(1-t), normalize ----
                den = smp.tile([128, SUB, PHASES], f32, tag="den")
                nc.vector.tensor_scalar(
                    out=den, in0=ppt, scalar1=-1.0, scalar2=1.0,
                    op0=ALU.mult, op1=ALU.add,
                )
                pw8 = smp.tile([128, SUB, NP8], f32, tag="pw8")
                nc.gpsimd.memset(pw8[:, :, PHASES : PHASES + 1], 1.0)
                nc.vector.reciprocal(out=den, in_=den)
                nc.vector.scalar_tensor_tensor(
                    out=pw8[:, :, :PHASES], in0=ppt, scalar=1.0, in1=den,
                    op0=ALU.add, op1=ALU.mult,
                )
                ssum = smp.tile([128, SUB], f32, tag="ssum")
                nc.vector.reduce_sum(
                    out=ssum, in_=pw8[:, :, :PHASES], axis=mybir.AxisListType.X
                )
                rec = smp.tile([128, SUB], f32, tag="rec")
                nc.vector.reciprocal(out=rec, in_=ssum)
                nc.vector.tensor_tensor(
                    out=pw8[:, :, :PHASES],
                    in0=pw8[:, :, :PHASES],
                    in1=rec[:, :, None].to_broadcast([128, SUB, PHASES]),
                    op=ALU.mult,
                )

                # ---- combine per 128-token subtile ----
                for s in range(SUB):
                    prod = prodp.tile([128, NHK, NP8], mmdt)
                    nc.vector.tensor_tensor(
                        out=prod,
                        in0=pes[s].rearrange("p (hk e) -> p hk e", e=NP8),
                        in1=pw8[:, s : s + 1, :].to_broadcast([128, NHK, NP8]),
                        op=ALU.mult,
                    )
                    nc.vector.reduce_sum(
                        out=osb[:, s, :], in_=prod, axis=mybir.AxisListType.X
                    )

                nc.sync.dma_start(ov[m], osb)

    nc.compile()
    return nc


def _host_weights(inp):
    """Rearrange weights into on-device layouts (host-side, one-time)."""
    f = np.float32
    wt = np.zeros((128, 5, TRUNK), f)
    for b in range(4):
        wt[:, b, :] = inp["trunk_w"][b * 128 : (b + 1) * 128]
    wt[:82, 4, :] = inp["trunk_w"][512:IN_DIM]

    wr = np.zeros((128, 4, PHASES), f)
    for b in range(4):
        wr[:, b, :] = inp["router_w"][b * 128 : (b + 1) * 128]
    rb2 = np.ascontiguousarray(inp["router_b"].reshape(PHASES, 1)) * 0.5

    # heads: col c = (h*20+k)*8 + p ; p<7 experts, p=7 base
    wh_full = np.zeros((TRUNK, NHK, NP8), f)
    dr_full = np.zeros((NHK, NP8), f)
    wh_full[:, :BINS, :PHASES] = np.transpose(inp["inst_exp_w"], (1, 2, 0))
    wh_full[:, BINS:, :PHASES] = np.transpose(inp["group_exp_w"], (1, 2, 0))
    wh_full[:, :BINS, PHASES] = inp["inst_base_w"]
    wh_full[:, BINS:, PHASES] = inp["group_base_w"]
    dr_full[:BINS, :PHASES] = inp["inst_exp_b"].T
    dr_full[BINS:, :PHASES] = inp["group_exp_b"].T
    dr_full[:BINS, PHASES] = inp["inst_base_b"]
    dr_full[BINS:, PHASES] = inp["group_base_b"]
    wh = wh_full.reshape(TRUNK, NCOL).reshape(2, 128, NCOL).transpose(1, 0, 2).copy()
    dr = dr_full.reshape(1, NCOL).copy()

    tb = np.ascontiguousarray(inp["trunk_b"].reshape(2, 128).T)
    return wt, wr, wh, dr, tb, rb2


def kernel(**inputs):
    global LAST_RESULT
    import sys

    if "/opt/trn_rl_repo" not in sys.path:
        sys.path.insert(0, "/opt/trn_rl_repo")
    from concourse.bass_utils import run_bass_kernel_spmd

    inp = {
        k: np.asarray(
            v, dtype=np.float32 if np.asarray(v).dtype != np.int32 else np.int32
        )
        for k, v in inputs.items()
    }

    if "nc" not in _BUILT:
        _BUILT["nc"] = _build_module()
    nc = _BUILT["nc"]

    wt, wr, wh, dr, tb, rb2 = _host_weights(inp)

    x_full = np.concatenate(
        [inp["h_t"], inp["a_t"], inp["d_t"], inp["age_embed"]], axis=-1
    )  # [B, T, 594]

    mmdt_name = _mm_dt_name()
    if mmdt_name == "bf16":
        import ml_dtypes

        cdt = ml_dtypes.bfloat16
        x_full = x_full.astype(cdt)
        wt, wr, wh, dr = (a.astype(cdt) for a in (wt, wr, wh, dr))

    in_maps = []
    for c in range(NCORES):
        xc = x_full[c * B_LOC : (c + 1) * B_LOC].reshape(NTOK, IN_DIM)
        xT = np.ascontiguousarray(xc.T)  # [594, NTOK] feature-major
        in_maps.append(
            {
                "xh": xT[:HID],
                "xr": xT[HID:],
                "wt": wt,
                "wr": wr,
                "wh": wh,
                "dr": dr,
                "tb": tb,
                "rb2": rb2,
            }
        )

    res = run_bass_kernel_spmd(nc, in_maps, core_ids=list(range(NCORES)))
    LAST_RESULT = res

    inst = np.empty((B, T, BINS), np.float32)
    grp = np.empty((B, T, BINS), np.float32)
    for c in range(NCORES):
        o = res.results[c]["out"].reshape(B_LOC, T, NHK)
        inst[c * B_LOC : (c + 1) * B_LOC] = o[:, :, :BINS]
        grp[c * B_LOC : (c + 1) * B_LOC] = o[:, :, BINS:]
    return inst, grp


# revision 35
# speedup vs baseline: 1.6640x; 1.1032x over previous
"""Trainium2 Bass kernel for DualHazardHead (moe_routing).

Computation per token t:
  x = concat(h, a, d, age)            [594]
  z = gelu(x @ Wt + bt)               [256]
  pw = softmax(h @ Wr + br)           [7]
  inst  = z @ Wbi + bbi + sum_p pw_p (z @ Wei_p + bei_p)   [20]
  group = z @ Wbg + bbg + sum_p pw_p (z @ Weg_p + beg_p)   [20]

Sharding: pure data parallel over B (32 -> 4 per core) on 8 NeuronCores.

v2 design (per core, NTOK=8192 tokens, 16 macro tiles of 512):
  - x is transposed on the HOST to feature-major [594, NTOK] (and cast to
    bf16), so the kernel DMAs xT blocks straight into SBUF: no on-device
    PE transposes, no PSUM->SBUF copies for x.
  - trunk zT [256, tok] in PSUM -> exact GELU on ACT (bias fused) -> bf16.
  - router logits pwT [7, tok] on PE; tanh(l/2 + rb/2) computed PHASE-major
    on ACT (router bias fused into the activation), then 4 small PE
    transposes to token-side; softmax exp via (1+t)/(1-t) on DVE so GELU
    and softmax share one ACT table set.
  - heads: pe [128tok, 320] PSUM; columns c=(h*20+k)*8+p with p in 0..6 =
    experts, p=7 = base head; biases are PRE-WRITTEN into the PSUM bank by
    ScalarE (drb, host-broadcast to 128 partitions) and the z matmuls run
    with start=False, accumulating on top (has_written bits stay set from
    the prewarm / previous macro).  Combine = broadcast multiply by pw8
    (slot 7 = 1.0) on DVE + strided reduce over p on GpSimd.
  - PE program order per macro: trunk(10) -> router(4) -> heads(8) ->
    softmax transposes(4), so no PE stall waits on the ACT tanh roundtrip.
"""

import os

import numpy as np

B, T = 32, 2048
HID, ACTD, SRC, AGE = 512, 64, 2, 16
TRUNK, BINS, PHASES = 256, 20, 7
IN_DIM = HID + ACTD + SRC + AGE  # 594
NCORES = 8
B_LOC = B // NCORES  # 4
NTOK = B_LOC * T  # 8192
MACRO = 512
NMACRO = NTOK // MACRO  # 16
SUB = MACRO // 128  # 4
NHK = 2 * BINS  # 40 (head, bin) pairs
NP8 = PHASES + 1  # 7 experts + 1 base slot
NCOL = NHK * NP8  # 320 head-matmul output columns
KBLK = [128, 128, 128, 128, 82]  # xT k-block sizes (594 features)

_BUILT = {}
LAST_RESULT = None


def _mm_dt_name():
    return os.environ.get("KERNEL_MM_DT", "bf16")


def _build_module():
    """Build the Bass module (same NEFF for all cores)."""
    import concourse.bass as bass
    import concourse.tile as tile
    from concourse import bacc, mybir
    from concourse.masks import make_identity

    f32 = mybir.dt.float32
    mmdt = {"f32": f32, "f32r": mybir.dt.float32r, "bf16": mybir.dt.bfloat16}[
        _mm_dt_name()
    ]

    AF = mybir.ActivationFunctionType
    ALU = mybir.AluOpType
    ts = bass.ts

    nc = bacc.Bacc("TRN2", target_bir_lowering=False, debug=False)

    xh_d = nc.dram_tensor("xh", [HID, NTOK], mmdt, kind="ExternalInput")
    xr_d = nc.dram_tensor("xr", [IN_DIM - HID, NTOK], mmdt, kind="ExternalInput")
    wt_d = nc.dram_tensor("wt", [128, 5, TRUNK], mmdt, kind="ExternalInput")
    wr_d = nc.dram_tensor("wr", [128, 4, PHASES], mmdt, kind="ExternalInput")
    wh_d = nc.dram_tensor("wh", [128, 2, NCOL], mmdt, kind="ExternalInput")
    drb_d = nc.dram_tensor("drb", [128, SUB, NCOL], f32, kind="ExternalInput")
    tb_d = nc.dram_tensor("tb", [128, 2], f32, kind="ExternalInput")
    rb2_d = nc.dram_tensor("rb2", [PHASES, 1], f32, kind="ExternalInput")
    out_d = nc.dram_tensor("out", [NTOK, NHK], f32, kind="ExternalOutput")

    xhv = xh_d[:, :].rearrange("(b p) (m t) -> m p b t", p=128, t=MACRO)
    xrv = xr_d[:, :].rearrange("q (m t) -> m q t", t=MACRO)
    ov = out_d[:, :].rearrange("(m s p) k -> m p s k", p=128, s=SUB)

    with tile.TileContext(nc) as tc:
        with (
            tc.tile_pool(name="const", bufs=1) as const,
            tc.tile_pool(name="xin", bufs=3) as xin,
            tc.tile_pool(name="zs", bufs=2) as zsp,
            tc.tile_pool(name="sm", bufs=2) as smp,
            tc.tile_pool(name="prod", bufs=3) as prodp,
            tc.tile_pool(name="outp", bufs=2) as outp,
            tc.tile_pool(name="ps_z", bufs=2, space="PSUM") as ps_z,
            tc.tile_pool(name="ps_pw", bufs=1, space="PSUM") as ps_pw,
            tc.tile_pool(name="ps_e", bufs=1, space="PSUM") as ps_e,
        ):
            ident_f = const.tile([128, 128], f32)
            make_identity(nc, ident_f)
            wt = const.tile([128, 5, TRUNK], mmdt)
            nc.gpsimd.dma_start(wt, wt_d[:])
            wr = const.tile([128, 4, PHASES], mmdt)
            nc.gpsimd.dma_start(wr, wr_d[:])
            wh = const.tile([128, 2, NCOL], mmdt)
            nc.gpsimd.dma_start(wh, wh_d[:])
            drb = const.tile([128, SUB, NCOL], f32)
            nc.gpsimd.dma_start(drb, drb_d[:])
            tb = const.tile([128, 2], f32)
            nc.gpsimd.dma_start(tb, tb_d[:])
            rb2 = const.tile([PHASES, 1], f32)
            nc.gpsimd.dma_start(rb2, rb2_d[:])

            # PE prewarm: consume each const via a cheap dummy matmul so later
            # real PE instructions never stack startup semaphore waits.  The
            # four pe-slot dummies also SET the per-element has_written bits
            # over the full [128, 320] region of every pe PSUM slot, so the
            # steady-state heads matmuls can run with start=False and
            # accumulate onto the ScalarE-prewritten bias (drb).
            pwf = ps_pw.tile([128, MACRO], f32, tag="ppw")
            nc.tensor.transpose(pwf[:7, 0:7], ident_f[:7, :7], ident_f[:7, :7])
            nc.tensor.matmul(
                pwf[:7, 0:128], wr[:, 0, :], wt[:, 0, 0:128],
                start=True, stop=True,
            )
            pe_w = ps_e.tile([128, SUB, MACRO], f32, tag="pe")
            for _s in range(SUB):
                nc.tensor.matmul(
                    pe_w[:, _s, 0:NCOL], wt[:, 0, 0:128], wh[:, 0, :],
                    start=True, stop=True,
                )

            for m in range(NMACRO):
                # ---- load xT (feature-major, straight from host) ----
                xt = xin.tile([128, 5, MACRO], mmdt)
                nc.sync.dma_start(xt[:, 0:4, :], xhv[m])
                nc.sync.dma_start(xt[: KBLK[4], 4, :], xrv[m])

                # ---- ScalarE pre-writes head biases into all 4 pe banks
                # (emitted first so it sits ahead of the GELUs in the ACT
                # queue and is long done before the z matmuls need it) ----
                petile = ps_e.tile([128, SUB, MACRO], f32, tag="pe")
                nc.scalar.copy(out=petile[:, :, 0:NCOL], in_=drb)

                # ---- trunk matmuls: zT [256, 512] over 2 PSUM halves ----
                pz0 = ps_z.tile([128, MACRO], f32, tag="pz")
                pz1 = ps_z.tile([128, MACRO], f32, tag="pz")
                for b in range(5):
                    kb = KBLK[b]
                    nc.tensor.matmul(
                        pz0, wt[:kb, b, 0:128], xt[:kb, b, :],
                        start=(b == 0), stop=(b == 4),
                    )
                    nc.tensor.matmul(
                        pz1, wt[:kb, b, 128:256], xt[:kb, b, :],
                        start=(b == 0), stop=(b == 4),
                    )

                # ---- router matmuls: pwT [7, 512] (h = blocks 0..3) ----
                ppw = ps_pw.tile([128, MACRO], f32, tag="ppw")
                for b in range(4):
                    nc.tensor.matmul(
                        ppw[:PHASES], wr[:128, b, :], xt[:128, b, :],
                        start=(b == 0), stop=(b == 3),
                    )

                # ---- GELU (exact) with fused trunk bias -> bf16 z ----
                zs = zsp.tile([128, 2, MACRO], mmdt)
                nc.scalar.activation(
                    out=zs[:, 0, :], in_=pz0, func=AF.Gelu,
                    bias=tb[:, 0:1], scale=1.0,
                )
                nc.scalar.activation(
                    out=zs[:, 1, :], in_=pz1, func=AF.Gelu,
                    bias=tb[:, 1:2], scale=1.0,
                )

                # ---- tanh(l/2 + rb/2) phase-major (router bias fused) ----
                thp = smp.tile([PHASES, MACRO], f32, tag="thp")
                nc.scalar.activation(
                    out=thp, in_=ppw[:PHASES], func=AF.Tanh,
                    bias=rb2, scale=0.5,
                )

                # ---- heads: petile[:, s, 0:320] per subtile ----
                # The z matmuls run with start=False and accumulate onto the
                # ScalarE-prewritten biases (has_written bits stay set from
                # the prewarm / previous macro, so the PE adds instead of
                # overwriting).
                osb = outp.tile([128, SUB, NHK], f32)
                for s in range(SUB):
                    nc.tensor.matmul(
                        petile[:, s, 0:NCOL], zs[:, 0, ts(s, 128)], wh[:, 0, :],
                        start=False, stop=False,
                    )
                    nc.tensor.matmul(
                        petile[:, s, 0:NCOL], zs[:, 1, ts(s, 128)], wh[:, 1, :],
                        start=False, stop=True,
                    )

                # ---- tanh to token-side (after heads in PE queue) ----
                # ppt aliases the ppw bank: ppw is dead once thp is computed.
                ppt = ps_pw.tile([128, SUB, PHASES], f32, tag="ppw")
                for s in range(SUB):
                    nc.tensor.transpose(
                        ppt[:, s, :], thp[:, ts(s, 128)],
                        ident_f[:PHASES, :PHASES],
                    )

                # ---- softmax from tanh: exp(l) = (1+t)/(1-t), normalize ----
                den = smp.tile([128, SUB, PHASES], f32, tag="den")
                nc.vector.tensor_scalar(
                    out=den, in0=ppt, scalar1=-1.0, scalar2=1.0,
                    op0=ALU.mult, op1=ALU.add,
                )
                pw8 = smp.tile([128, SUB, NP8], f32, tag="pw8")
                nc.gpsimd.memset(pw8[:, :, PHASES : PHASES + 1], 1.0)
                nc.vector.reciprocal_approx_fast(out=den, in_=den)
                nc.vector.scalar_tensor_tensor(
                    out=pw8[:, :, :PHASES], in0=ppt, scalar=1.0, in1=den,
                    op0=ALU.add, op1=ALU.mult,
                )
                ssum = smp.tile([128, SUB], f32, tag="ssum")
                nc.vector.reduce_sum(
                    out=ssum, in_=pw8[:, :, :PHASES], axis=mybir.AxisListType.X
                )
                rec = smp.tile([128, SUB], f32, tag="rec")
                nc.vector.reciprocal_approx_fast(out=rec, in_=ssum)
                nc.vector.tensor_tensor(
                    out=pw8[:, :, :PHASES],
                    in0=pw8[:, :, :PHASES],
                    in1=rec[:, :, None].to_broadcast([128, SUB, PHASES]),
                    op=ALU.mult,
                )

                # ---- combine: ONE multiply + ONE reduce over all 4 banks ----
                prod = prodp.tile([128, SUB, NHK, NP8], mmdt)
                nc.vector.tensor_tensor(
                    out=prod,
                    in0=petile[:, :, 0:NCOL].rearrange(
                        "p s (hk e) -> p s hk e", e=NP8
                    ),
                    in1=pw8[:, :, None, :].to_broadcast([128, SUB, NHK, NP8]),
                    op=ALU.mult,
                )
                nc.vector.reduce_sum(
                    out=osb, in_=prod, axis=mybir.AxisListType.X
                )

                nc.sync.dma_start(ov[m], osb)

    nc.compile()
    return nc


def _host_weights(inp):
    """Rearrange weights into on-device layouts (host-side, one-time)."""
    f = np.float32
    wt = np.zeros((128, 5, TRUNK), f)
    for b in range(4):
        wt[:, b, :] = inp["trunk_w"][b * 128 : (b + 1) * 128]
    wt[:82, 4, :] = inp["trunk_w"][512:IN_DIM]

    wr = np.zeros((128, 4, PHASES), f)
    for b in range(4):
        wr[:, b, :] = inp["router_w"][b * 128 : (b + 1) * 128]
    rb2 = np.ascontiguousarray(inp["router_b"].reshape(PHASES, 1)) * 0.5

    # heads: col c = (h*20+k)*8 + p ; p<7 experts, p=7 base
    wh_full = np.zeros((TRUNK, NHK, NP8), f)
    dr_full = np.zeros((NHK, NP8), f)
    wh_full[:, :BINS, :PHASES] = np.transpose(inp["inst_exp_w"], (1, 2, 0))
    wh_full[:, BINS:, :PHASES] = np.transpose(inp["group_exp_w"], (1, 2, 0))
    wh_full[:, :BINS, PHASES] = inp["inst_base_w"]
    wh_full[:, BINS:, PHASES] = inp["group_base_w"]
    dr_full[:BINS, :PHASES] = inp["inst_exp_b"].T
    dr_full[BINS:, :PHASES] = inp["group_exp_b"].T
    dr_full[:BINS, PHASES] = inp["inst_base_b"]
    dr_full[BINS:, PHASES] = inp["group_base_b"]
    wh = wh_full.reshape(TRUNK, NCOL).reshape(2, 128, NCOL).transpose(1, 0, 2).copy()
    drb = np.ascontiguousarray(
        np.broadcast_to(dr_full.reshape(1, 1, NCOL), (128, SUB, NCOL))
    )

    tb = np.ascontiguousarray(inp["trunk_b"].reshape(2, 128).T)
    return wt, wr, wh, drb, tb, rb2


def kernel(**inputs):
    global LAST_RESULT
    import sys

    if "/opt/trn_rl_repo" not in sys.path:
        sys.path.insert(0, "/opt/trn_rl_repo")
    from concourse.bass_utils import run_bass_kernel_spmd

    inp = {
        k: np.asarray(
            v, dtype=np.float32 if np.asarray(v).dtype != np.int32 else np.int32
        )
        for k, v in inputs.items()
    }

    if "nc" not in _BUILT:
        _BUILT["nc"] = _build_module()
    nc = _BUILT["nc"]

    wt, wr, wh, drb, tb, rb2 = _host_weights(inp)

    x_full = np.concatenate(
        [inp["h_t"], inp["a_t"], inp["d_t"], inp["age_embed"]], axis=-1
    )  # [B, T, 594]

    mmdt_name = _mm_dt_name()
    if mmdt_name == "bf16":
        import ml_dtypes

        cdt = ml_dtypes.bfloat16
        x_full = x_full.astype(cdt)
        wt, wr, wh = (a.astype(cdt) for a in (wt, wr, wh))

    in_maps = []
    for c in range(NCORES):
        xc = x_full[c * B_LOC : (c + 1) * B_LOC].reshape(NTOK, IN_DIM)
        xT = np.ascontiguousarray(xc.T)  # [594, NTOK] feature-major
        in_maps.append(
            {
                "xh": xT[:HID],
                "xr": xT[HID:],
                "wt": wt,
                "wr": wr,
                "wh": wh,
                "drb": drb,
                "tb": tb,
                "rb2": rb2,
            }
        )

    res = run_bass_kernel_spmd(nc, in_maps, core_ids=list(range(NCORES)))
    LAST_RESULT = res

    inst = np.empty((B, T, BINS), np.float32)
    grp = np.empty((B, T, BINS), np.float32)
    for c in range(NCORES):
        o = res.results[c]["out"].reshape(B_LOC, T, NHK)
        inst[c * B_LOC : (c + 1) * B_LOC] = o[:, :, :BINS]
        grp[c * B_LOC : (c + 1) * B_LOC] = o[:, :, BINS:]
    return inst, grp
